# revision 37
# baseline (speedup 1.0000x reference)
"""Trainium2 Bass kernel for nn_DeltaResidualExpanded.

Computes, per (b, t) position:
    k    = l2normalize(sublayer_output) / sqrt(D)
    beta = 2*sigmoid(RMSNorm(x_in) @ gate_w.T + gate_b)
    v    = x_in @ Wv.T
    out  = X + beta * k (outer) (v - k.X)

Pure data-parallel over B*T across 8 NeuronCores; each core streams its
1024 positions as 8 tiles of 128 partitions. Inputs/outputs are staged
bf16 on the host (rel tolerance 2e-2 >> bf16's ~4e-3), and X is staged
j-major ([pos, j, d]) so every per-j DVE op streams contiguously:
~36 MiB HBM traffic/core (~105 us DMA floor at ~358 GB/s/core).

Engine layout per tile (~15 us/tile on each of DVE and ACT, balanced):
  - SP(sync):  SX chunks interleaved with 2 MiB X tile loads, and the
               1 MiB j-group OUT stores (loads run ~7 tiles ahead so
               store waits never starve them)
  - PE:        xin^T transposes (into one PSUM tile) + matmul against
               [Wv.T | gate_norm_w*gate_w] -> v and gate logit
  - ACT:       Square-accum norms, sqrt, sigmoid, one wide PSUM drain,
               and the 8 per-tile prescales usub_j = sub*corr2[j]
               (Copy with per-partition scale)
  - DVE:       8 fused mult+reduce scalar_tensor_tensor ops (raw_j =
               sub . X_j, 1x mode), per-position algebra, and one
               batched bf16 2x-mode tensor_tensor add per j-group
               (X_g += usub4)
  - GPSIMD:    nothing (its compute ops stall DVE ~10x via SBUF
               contention; its tensor_scalar ucode produces NaN on HW)

ACT table-set discipline: Square+Sqrt era then Sigmoid era per tile
group, so table loads are per-group instead of 2 per tile. Emission is
grouped (ph1/ph2 per tile group) so early tiles' phase 2 is not stuck
behind late tiles' phase 1 in the in-order engine streams; the last
tile takes the stt-update path with per-j-pair stores to shorten the
final drain.

The walrus build in this container accepts at most ONE on_wait condition
per instruction, so the Tile-scheduled BIR is post-processed to hoist
extra waits into standalone EventSemaphore instructions (legalize_bir).
"""
import sys
import math

sys.path.insert(0, "/opt/trn_rl_repo")

import numpy as np

B, T, D, DV = 4, 2048, 1024, 8
N_CORES = 8
BT = B * T
CORE_BT = BT // N_CORES          # 1024 positions per core
P = 128                          # partitions per tile
NT = CORE_BT // P                # 8 tiles per core
NC_D = D // P                    # 8 d-chunks of 128
HD = D // 2
JG = 4                           # j-group size for the GPSIMD product
NJG = DV // JG
EPS_K = 1e-6
EPS_NORM = 1e-6
W_COLS = DV + 2                  # Wv rows, gate row, zero pad

# pre[t] column layout ([P, 64] f32 per tile)
C_RAW = 0       # 0..7   raw[j] = sub . X[:, :, j]
C_CORR2 = 8     # 8..15  corr2[j] = A*v[j] - B*raw[j]
C_VGT = 16      # 16..25 [v(8) | g | pad]
C_AV = 26       # 26..33 A*v
C_SSQ = 34
C_XSQ = 35
C_SSQC = 36     # max(ssq, EPS_K^2); must be adjacent to C_XSQP
C_XSQP = 37     # xsq/D + EPS_NORM
C_SQ = 38       # 38..39 sqrt of [36..37] = [snorm, rmsden]
C_RCP = 40      # 40..41 recip of [38..39] = [sinv, rms]
C_LOGIT = 42
C_SIG = 43
C_A = 44        # 2*sig*sinv/sqrt(D)
C_NEGB = 45     # -A*sinv/sqrt(D)

_NC_CACHE: dict = {}


def legalize_bir_dict(d):
    """Split multi-wait instructions (this walrus accepts one on_wait per
    instruction): hoist extras into standalone EventSemaphore instrs."""
    n = 0
    for fn in d.get("functions", []):
        for blk in fn.get("blocks", []):
            insts = blk.get("instructions")
            if not insts:
                continue
            out = []
            for inst in insts:
                si = inst.get("sync_info")
                waits = (si or {}).get("on_wait") or []
                if len(waits) > 1:
                    for w in waits[:-1]:
                        n += 1
                        out.append({
                            "debug": inst.get("debug", 0),
                            "engine": inst["engine"],
                            "ins": [],
                            "name": f"legwait-{n}",
                            "opcode": "EventSemaphore",
                            "outs": [],
                            "sync_info": {"on_update": [], "on_wait": [w]},
                        })
                    si["on_wait"] = waits[-1:]
                out.append(inst)
            blk["instructions"] = out
    return d


def _build(gate_b_val: float, opts: dict | None = None):
    opts = dict(opts or {})
    bf16_io = opts.get("bf16_io", True)
    xbufs = opts.get("xbufs", 7)     # 2 MiB x tiles in flight
    upre = opts.get("upre", True)   # updates via ACT prescale + DVE TT add
    tsacc = opts.get("tsacc", False)  # reduce via TT-prod + ts-accum (slower: ts-accum is 1x)

    import orjson
    import concourse.bass as bass
    import concourse.tile as tile
    from concourse import mybir, masks
    from concourse.bass import ts
    from contextlib import ExitStack

    f32 = mybir.dt.float32
    bf16 = mybir.dt.bfloat16
    xdt = bf16 if bf16_io else f32
    AF = mybir.ActivationFunctionType
    OP = mybir.AluOpType

    nc = bass.Bass()
    # X is staged j-major on the host: [pos, j, d] so every per-j DVE op
    # (reduce and update) streams contiguously instead of stride-8
    X = nc.dram_tensor("X", [CORE_BT, DV, D], xdt, kind="ExternalInput")
    # SX = [sublayer_output | x_in] fused along the feature axis
    SX = nc.dram_tensor("SX", [CORE_BT, 2 * D], xdt, kind="ExternalInput")
    # [D, W_COLS]: cols 0..7 = Wv.T, col 8 = gate_norm_w*gate_w, col 9 = 0
    WT = nc.dram_tensor("WT", [D, W_COLS], xdt, kind="ExternalInput")
    OUT = nc.dram_tensor("OUT", [CORE_BT, DV, D], xdt,
                         kind="ExternalOutput")

    with tile.TileContext(nc) as tc, ExitStack() as ctx:
        consts = ctx.enter_context(tc.tile_pool(name="consts", bufs=1))
        xpool = ctx.enter_context(tc.tile_pool(name="xpool", bufs=xbufs))
        xtp = ctx.enter_context(tc.tile_pool(name="xtp", bufs=3))
        prep = ctx.enter_context(tc.tile_pool(name="prep", bufs=NT))
        vgsp = ctx.enter_context(tc.tile_pool(name="vgsp", bufs=2))
        usubp = ctx.enter_context(tc.tile_pool(name="usubp", bufs=5))
        prodp = ctx.enter_context(tc.tile_pool(name="prodp", bufs=3))
        tpsum = ctx.enter_context(tc.tile_pool(name="tpsum", bufs=3,
                                               space="PSUM"))
        vpsum = ctx.enter_context(tc.tile_pool(name="vpsum", bufs=2,
                                               space="PSUM"))
        wpsum = ctx.enter_context(tc.tile_pool(name="wpsum", bufs=2,
                                               space="PSUM"))

        ident = consts.tile([P, P], f32)
        masks.make_identity(nc, ident[:])
        identx = consts.tile([P, P], xdt)
        masks.make_identity(nc, identx[:])
        # WT load as [128 d-in-chunk, chunk, col]
        wt_sb = consts.tile([P, NC_D, W_COLS], xdt)
        nc.gpsimd.dma_start(
            out=wt_sb, in_=WT[:].rearrange("(c p) m -> p c m", p=P))
        # shared throwaway outputs for accumulate ops; one per engine so
        # the WAW dependency stays same-engine (no cross-engine sync)
        scrb = consts.tile([P, D], bf16)
        scrd = consts.tile([P, D], bf16)
        # all 8 SX tiles as [p, tile, col]; loaded in two halves so the
        # first X tiles slot between them on the SP DMA ring
        sx_all = consts.tile([P, NT, 2 * D], xdt)
        sx_r = SX[:].rearrange("(t p) m -> p t m", p=P)
        HT = NT // 2

        pres = []
        x_gran = {}

        # ---- DMA issue order on the SP ring: SX interleaved with X
        # loads in 2-tile chunks: SX01, X0, SX23, X1, SX45, X2, SX67, ...
        sx_chunks = [(0, 1), (1, 2), (2, 4), (4, 6), (6, 8)]
        for t in range(NT):
            if t < len(sx_chunks):
                lo, hi = sx_chunks[t]
                nc.sync.dma_start(out=sx_all[:, lo:hi, :],
                                  in_=sx_r[:, lo:hi, :])
            x_t = xpool.tile([P, DV, D], xdt, tag="xg")
            nc.sync.dma_start(out=x_t, in_=X[t * P:(t + 1) * P])
            for g in range(NJG):
                x_gran[(t, g)] = x_t[:, g * JG:(g + 1) * JG, :]

        # ---- phase 1, two ACT-table eras per half: norms (Square+
        # Sqrt set), then gate (Sigmoid set) -- so table loads are
        # per-half, not per-tile
        def emit_ph1_norms(t):
            sub = sx_all[:, t, 0:D]
            xin = sx_all[:, t, D:2 * D]
            pre = prep.tile([P, 64], f32)
            pres.append(pre)

            # norms: ssq = sum(sub^2), xsq = sum(xin^2)
            nc.scalar.activation(out=scrb, in_=sub, func=AF.Square,
                                 accum_out=pre[:, C_SSQ:C_SSQ + 1])
            nc.scalar.activation(out=scrb, in_=xin, func=AF.Square,
                                 accum_out=pre[:, C_XSQ:C_XSQ + 1])
            nc.vector.tensor_scalar_max(out=pre[:, C_SSQC:C_SSQC + 1],
                                        in0=pre[:, C_SSQ:C_SSQ + 1],
                                        scalar1=EPS_K * EPS_K)
            nc.vector.tensor_scalar(out=pre[:, C_XSQP:C_XSQP + 1],
                                    in0=pre[:, C_XSQ:C_XSQ + 1],
                                    scalar1=1.0 / D, scalar2=EPS_NORM,
                                    op0=OP.mult, op1=OP.add)
            nc.scalar.activation(out=pre[:, C_SQ:C_SQ + 2],
                                 in_=pre[:, C_SSQC:C_SSQC + 2],
                                 func=AF.Sqrt)
            nc.vector.reciprocal(out=pre[:, C_RCP:C_RCP + 2],
                                 in_=pre[:, C_SQ:C_SQ + 2])

        def emit_ph1_gate(t):
            pre = pres[t]
            xin = sx_all[:, t, D:2 * D]
            # v & gate dot via PE: xin^T chunks into ONE PSUM tile,
            # drained by a single wide ACT copy
            xt_sb = xtp.tile([P, NC_D, P], xdt)
            ps8 = tpsum.tile([P, NC_D, P], xdt, tag="tp")
            for c in range(NC_D):
                nc.tensor.transpose(ps8[:, c, :], xin[:, ts(c, P)],
                                    identx[:])
            nc.scalar.copy(out=xt_sb[:], in_=ps8[:])
            vg_ps = vpsum.tile([W_COLS, P], f32, tag="vg")
            for c in range(NC_D):
                nc.tensor.matmul(vg_ps[:, :], wt_sb[:, c, :],
                                 xt_sb[:, c, :],
                                 start=(c == 0), stop=(c == NC_D - 1))
            vg_sb = vgsp.tile([W_COLS, P], f32, tag="vgsb")
            nc.scalar.copy(out=vg_sb[:], in_=vg_ps[:])
            vgt_ps = wpsum.tile([P, W_COLS], f32, tag="vgt")
            nc.tensor.transpose(vgt_ps[:], vg_sb[:],
                                ident[:W_COLS, :W_COLS])
            nc.scalar.copy(out=pre[:, C_VGT:C_VGT + W_COLS], in_=vgt_ps[:])

            # logit = g*rms + gate_b
            nc.vector.tensor_scalar(out=pre[:, C_LOGIT:C_LOGIT + 1],
                                    in0=pre[:, C_VGT + DV:C_VGT + DV + 1],
                                    scalar1=pre[:, C_RCP + 1:C_RCP + 2],
                                    scalar2=gate_b_val,
                                    op0=OP.mult, op1=OP.add)
            nc.scalar.activation(out=pre[:, C_SIG:C_SIG + 1],
                                 in_=pre[:, C_LOGIT:C_LOGIT + 1],
                                 func=AF.Sigmoid)
            nc.vector.tensor_scalar(out=pre[:, C_A:C_A + 1],
                                    in0=pre[:, C_SIG:C_SIG + 1],
                                    scalar1=pre[:, C_RCP:C_RCP + 1],
                                    scalar2=2.0 / math.sqrt(D),
                                    op0=OP.mult, op1=OP.mult)
            nc.vector.tensor_scalar(out=pre[:, C_NEGB:C_NEGB + 1],
                                    in0=pre[:, C_A:C_A + 1],
                                    scalar1=pre[:, C_RCP:C_RCP + 1],
                                    scalar2=-1.0 / math.sqrt(D),
                                    op0=OP.mult, op1=OP.mult)
            nc.vector.tensor_scalar_mul(out=pre[:, C_AV:C_AV + DV],
                                        in0=pre[:, C_VGT:C_VGT + DV],
                                        scalar1=pre[:, C_A:C_A + 1])

        # ---- phase 2: X-dependent work, per 2 MiB tile / 1 MiB j-group
        def emit_ph2(t):
            rows = slice(t * P, (t + 1) * P)
            pre = pres[t]
            sub = sx_all[:, t, 0:D]
            # both granules' reduces + corr2 first, so DVE never blocks
            # behind the ACT prescales of the first granule
            for g in range(NJG):
                xg = x_gran[(t, g)]
                for jj in range(JG):
                    j = g * JG + jj
                    if tsacc:
                        # 2x-mode bf16 product, then single-src
                        # tensor_scalar accumulate (higher perf mode)
                        pr = prodp.tile([P, D], xdt, tag="pr")
                        nc.vector.tensor_tensor(out=pr[:], in0=xg[:, jj, :],
                                                in1=sub, op=OP.mult)
                        nc.vector.tensor_scalar(
                            out=scrd, in0=pr[:], scalar1=1.0, scalar2=0.0,
                            op0=OP.mult, op1=OP.add,
                            accum_out=pre[:, C_RAW + j:C_RAW + j + 1])
                    else:
                        nc.vector.scalar_tensor_tensor(
                            out=scrd, in0=xg[:, jj, :], scalar=1.0, in1=sub,
                            op0=OP.mult, op1=OP.mult,
                            accum_out=pre[:, C_RAW + j:C_RAW + j + 1])
                # corr2[j in group] = raw*negB + A*v
                nc.vector.scalar_tensor_tensor(
                    out=pre[:, C_CORR2 + g * JG:C_CORR2 + (g + 1) * JG],
                    in0=pre[:, C_RAW + g * JG:C_RAW + (g + 1) * JG],
                    scalar=pre[:, C_NEGB:C_NEGB + 1],
                    in1=pre[:, C_AV + g * JG:C_AV + (g + 1) * JG],
                    op0=OP.mult, op1=OP.add)
            for g in range(NJG):
                xg = x_gran[(t, g)]
                jsl = slice(g * JG, (g + 1) * JG)
                if upre and t != NT - 1:
                    # usub4[jj] = sub*corr2[j] on ACT (per-partition
                    # scale), then ONE batched 2x-mode bf16 add on DVE
                    us4 = usubp.tile([P, JG, D], xdt, tag="us")
                    for jj in range(JG):
                        j = g * JG + jj
                        nc.scalar.activation(
                            out=us4[:, jj, :], in_=sub, func=AF.Copy,
                            scale=pre[:, C_CORR2 + j:C_CORR2 + j + 1])
                    nc.vector.tensor_tensor(
                        out=xg, in0=xg, in1=us4[:], op=OP.add)
                    # stores alternate across the SP and ACT HWDGE
                    # rings so consecutive stores (and the final drain)
                    # overlap instead of queueing on one FIFO
                    eng = nc.sync if g % 2 == 0 else nc.scalar
                    eng.dma_start(out=OUT[rows, jsl], in_=xg)
                else:
                    # last tile: stt updates + per-j-pair stores so the
                    # final drain starts as early as possible
                    for jj in range(JG):
                        j = g * JG + jj
                        nc.vector.scalar_tensor_tensor(
                            out=xg[:, jj, :], in0=sub,
                            scalar=pre[:, C_CORR2 + j:C_CORR2 + j + 1],
                            in1=xg[:, jj, :], op0=OP.mult, op1=OP.add)
                        if jj % 2 == 1:
                            j2 = slice(g * JG + jj - 1, g * JG + jj + 1)
                            eng = nc.sync if jj == 1 else nc.scalar
                            eng.dma_start(out=OUT[rows, j2],
                                          in_=x_gran[(t, g)][:, jj - 1:jj + 1, :])

        # emit in groups so early tiles' phase 2 is not stuck behind
        # late tiles' phase 1 in each engine's in-order stream; the
        # first group is small so the pipeline starts early
        groups = [(0, 2), (2, 4), (4, 8)]
        for lo, hi in groups:
            for t in range(lo, hi):
                emit_ph1_norms(t)
            for t in range(lo, hi):
                emit_ph1_gate(t)
            for t in range(lo, hi):
                emit_ph2(t)

    legal = orjson.dumps(legalize_bir_dict(nc.to_json()))
    nc.to_json_bytes = lambda: legal  # consumed by bass2jax custom-call
    return nc


def get_nc(gate_b_val: float, opts: dict | None = None):
    key = (float(gate_b_val), tuple(sorted((opts or {}).items())))
    if key not in _NC_CACHE:
        _NC_CACHE[key] = _build(gate_b_val, opts)
    return _NC_CACHE[key]


def make_in_maps(X, sublayer_output, x_in, gate_norm_w, gate_w, Wv,
                 bf16_io=True):
    import ml_dtypes
    sdt = ml_dtypes.bfloat16 if bf16_io else np.float32
    # stage X j-major: [pos, j, d] (same one-pass host copy as a reshape)
    Xf = np.asarray(X, dtype=np.float32).reshape(
        BT, D, DV).transpose(0, 2, 1).astype(sdt)
    SXf = np.concatenate(
        [np.asarray(sublayer_output, dtype=np.float32).reshape(BT, D),
         np.asarray(x_in, dtype=np.float32).reshape(BT, D)],
        axis=1).astype(sdt)
    gw = (np.asarray(gate_w, dtype=np.float32).reshape(D)
          * np.asarray(gate_norm_w, dtype=np.float32).reshape(D))
    WTv = np.zeros((D, W_COLS), dtype=np.float32)
    WTv[:, :DV] = np.asarray(Wv, dtype=np.float32).T
    WTv[:, DV] = gw
    WTv = WTv.astype(sdt)
    in_maps = []
    for c in range(N_CORES):
        sl = slice(c * CORE_BT, (c + 1) * CORE_BT)
        in_maps.append({"X": Xf[sl], "SX": SXf[sl], "WT": WTv})
    return in_maps


def kernel(X, sublayer_output, x_in, gate_norm_w, gate_w, gate_b, Wv):
    from concourse.bass_utils import run_bass_kernel_spmd

    gate_b_val = float(np.asarray(gate_b).reshape(-1)[0])
    nc = get_nc(gate_b_val)
    in_maps = make_in_maps(X, sublayer_output, x_in, gate_norm_w, gate_w, Wv)
    res = run_bass_kernel_spmd(nc, in_maps, list(range(N_CORES)))
    out = np.concatenate([res.results[c]["OUT"] for c in range(N_CORES)],
                         axis=0)
    # un-permute from the j-major device layout, upcast to f32
    return out.reshape(B, T, DV, D).transpose(0, 1, 3, 2).astype(np.float32)
